# revision 13
# baseline (speedup 1.0000x reference)
"""ExaoneMoESparseMoEBlock Trainium2 kernel.

Strategy (expert-parallel over 8 NeuronCores):
  - Routing (gate matmul + biased grouped top-k) computed host-side in float64
    (selection margins >> fp32 noise, matches the fp32 jax reference).
  - Tokens are dispatched host-side. Experts are assigned to (core, slot) by
    token-count rank in a serpentine order so that every core's slot s holds a
    similarly-loaded expert: slot capacities are uniform across cores (SPMD)
    and per-core work is balanced.
  - Each core runs the SiLU-gated MLP for its 8 experts over their gathered
    tokens in a transposed layout ([feature, token]) — weights stationary,
    tokens moving — so no on-device transposes are needed.
  - Matmul operands are fp16 (1 cyc/row at any N, ~10-bit mantissa; weights
    have sigma=0.02 so fp16 quantization error ~5e-4 relative), accumulation
    in fp32 PSUM. fp16 halves the weight-streaming bytes, which is the
    bandwidth floor of this kernel. Matmuls run at exact per-slot token
    widths. Expert outputs are written back in fp16 and upcast on host.
  - Weights stream in half-expert tiles (16KB/partition) through an 8-deep
    pool so the DMA queue never stalls on a slot release.
  - The shared expert (IS=2048) is tensor-parallel sharded over the 8 cores
    (256 inter-dim slice each); each core emits a full [H, T] partial.
  - Host applies routing weights, scatter-adds expert outputs, and sums the
    shared partials.
"""

import sys
import types

import numpy as np

T, H, E, K_TOP = 1024, 2048, 64, 8
G, TG = 8, 4
I_DIM, IS_DIM = 1024, 2048
SCALE = 2.5
N_CORES = 8
EPC = E // N_CORES       # experts per core
ISC = IS_DIM // N_CORES  # shared-expert intermediate slice per core
HC = H // 128            # 16 h-chunks
IC = I_DIM // 128        # 8 i-chunks
CMAX = 512               # hard per-expert capacity limit (moving-dim max)

_LAST_RESULT = None      # BassKernelResults of the most recent run (for test.py)


def _install_ntff_shim():
    """Register the axon NTFF profile hook if the image's antenv lacks it.

    Lets BASS_TRACE=1 produce a perfetto trace + exec_time_ns. Harmless no-op
    when tracing is off or the axon .so is absent.
    """
    try:
        import antenv
        if "antenv.axon_hooks" in sys.modules:
            return
        mod = types.ModuleType("antenv.axon_hooks")
        mod._hook = None
        mod.set_axon_ntff_profile_hook = lambda h: setattr(mod, "_hook", h)
        mod.get_axon_ntff_profile_hook = lambda: mod._hook
        sys.modules["antenv.axon_hooks"] = mod
        antenv.axon_hooks = mod
        from trn_agent_boot.trn_boot import _ntff_profile_via_ctypes
        mod.set_axon_ntff_profile_hook(
            _ntff_profile_via_ctypes("/opt/axon/libaxon_pjrt.so")
        )
    except Exception:
        pass


def _routing(x, gate_w, e_bias):
    """float64 replica of the reference's sigmoid biased grouped top-k."""
    logits = x.astype(np.float64) @ gate_w.astype(np.float64)
    scores = 1.0 / (1.0 + np.exp(-logits))
    sb = scores + e_bias.astype(np.float64)[None, :]
    gsz = E // G
    gs = sb.reshape(T, G, gsz)
    top2 = np.sort(gs, axis=-1)[:, :, -2:].sum(-1)
    gidx = np.argsort(-top2, axis=-1, kind="stable")[:, :TG]
    gmask = np.zeros((T, G), bool)
    gmask[np.arange(T)[:, None], gidx] = True
    masked = np.where(np.repeat(gmask, gsz, axis=1), sb, -np.inf)
    idx = np.argsort(-masked, axis=-1, kind="stable")[:, :K_TOP]
    w = np.take_along_axis(scores, idx, axis=1).astype(np.float32)
    w = w / w.sum(-1, keepdims=True)
    return (w * np.float32(SCALE)).astype(np.float32), idx.astype(np.int64)


_KERNEL_CACHE = {}


def _build_kernel(caps):
    """Per-core SPMD Bass program. caps[s] = token columns of expert slot s."""
    from concourse import bacc
    import concourse.mybir as mybir
    import concourse.tile as tile

    F32 = mybir.dt.float32
    F16 = mybir.dt.float16
    ACT = mybir.ActivationFunctionType

    nc = bacc.Bacc("TRN2", target_bir_lowering=False, debug=False)

    xe_d = [nc.dram_tensor(f"xe{s}", [128, HC, caps[s]], F16,
                           kind="ExternalInput") for s in range(EPC)]
    wg_d = nc.dram_tensor("wg", [EPC, HC, 128, I_DIM], F16, kind="ExternalInput")
    wu_d = nc.dram_tensor("wu", [EPC, HC, 128, I_DIM], F16, kind="ExternalInput")
    wd_d = nc.dram_tensor("wd", [EPC, IC, 128, H], F16, kind="ExternalInput")
    xt_d = nc.dram_tensor("xt", [HC, 128, T], F16, kind="ExternalInput")
    wsg_d = nc.dram_tensor("wsg", [HC, 128, ISC], F16, kind="ExternalInput")
    wsu_d = nc.dram_tensor("wsu", [HC, 128, ISC], F16, kind="ExternalInput")
    wsd_d = nc.dram_tensor("wsd", [ISC // 128, 128, H], F16, kind="ExternalInput")
    yr_d = [nc.dram_tensor(f"yr{s}", [128, HC, caps[s]], F16,
                           kind="ExternalOutput") for s in range(EPC)]
    ys_d = nc.dram_tensor("ys", [HC, 128, T], F16, kind="ExternalOutput")

    with tile.TileContext(nc) as tc:
        with (
            tc.tile_pool(name="wpool", bufs=8) as wpool,     # 16KB/part slots
            tc.tile_pool(name="xpool", bufs=2) as xpool,
            tc.tile_pool(name="sgpool", bufs=2) as sgpool,
            tc.tile_pool(name="apool", bufs=2) as apool,
            tc.tile_pool(name="opool", bufs=2) as opool,
            tc.tile_pool(name="xtpool", bufs=3) as xtpool,
            tc.tile_pool(name="pp", bufs=8, space="PSUM") as pp,
        ):
            # ------------- shared expert (TP slice of IS), single x pass -----
            wsg_t = wpool.tile([128, HC, ISC], F16, tag="w")
            nc.sync.dma_start(wsg_t[:], wsg_d.ap().rearrange("c p i -> p c i"))
            wsu_t = wpool.tile([128, HC, ISC], F16, tag="w")
            nc.sync.dma_start(wsu_t[:], wsu_d.ap().rearrange("c p i -> p c i"))
            psg_s = [pp.tile([128, 512], F32, name="ps", tag="ps")
                     for _ in range(4)]
            psu_s = [pp.tile([128, 512], F32, name="ps", tag="ps")
                     for _ in range(4)]
            for hc in range(HC):
                xt_t = xtpool.tile([128, T], F16)
                nc.sync.dma_start(xt_t[:], xt_d.ap()[hc])
                for it in range(2):
                    for nh in range(2):
                        nc.tensor.matmul(
                            psg_s[2 * it + nh][:],
                            wsg_t[:, hc, it * 128:(it + 1) * 128],
                            xt_t[:, nh * 512:(nh + 1) * 512],
                            start=(hc == 0), stop=(hc == HC - 1),
                        )
                        nc.tensor.matmul(
                            psu_s[2 * it + nh][:],
                            wsu_t[:, hc, it * 128:(it + 1) * 128],
                            xt_t[:, nh * 512:(nh + 1) * 512],
                            start=(hc == 0), stop=(hc == HC - 1),
                        )
            sg_s = sgpool.tile([128, 2, T], F32, tag="sg")
            sa_s = apool.tile([128, 2, T], F16, tag="a")
            for it in range(2):
                for nh in range(2):
                    sl = slice(nh * 512, (nh + 1) * 512)
                    nc.scalar.activation(
                        sg_s[:, it, sl], psg_s[2 * it + nh][:], ACT.Silu)
                    nc.vector.tensor_mul(
                        sa_s[:, it, sl], sg_s[:, it, sl], psu_s[2 * it + nh][:])

            wsd_t = wpool.tile([128, ISC // 128, H], F16, tag="w")
            nc.sync.dma_start(wsd_t[:], wsd_d.ap().rearrange("c p i -> p c i"))
            for htg in range(4):
                yo = opool.tile([128, 4, T], F16, tag="o")
                for hi in range(4):
                    ht = htg * 4 + hi
                    psy_s = [pp.tile([128, 512], F32, name="ps", tag="ps")
                             for _ in range(2)]
                    for ic in range(2):
                        for nh in range(2):
                            nc.tensor.matmul(
                                psy_s[nh][:],
                                wsd_t[:, ic, ht * 128:(ht + 1) * 128],
                                sa_s[:, ic, nh * 512:(nh + 1) * 512],
                                start=(ic == 0), stop=(ic == 1),
                            )
                    nc.vector.tensor_copy(yo[:, hi, 0:512], psy_s[0][:])
                    nc.vector.tensor_copy(yo[:, hi, 512:1024], psy_s[1][:])
                nc.sync.dma_start(
                    ys_d.ap().rearrange("c p t -> p c t")[:, htg * 4:(htg + 1) * 4, :],
                    yo[:])

            # ------------- routed experts -------------
            for e in range(EPC):
                cap = caps[e]
                xe_t = xpool.tile([128, HC, cap], F16, tag="xe")
                nc.sync.dma_start(xe_t[:], xe_d[e].ap())
                sg_t = sgpool.tile([128, IC, cap], F32, tag="sg")
                a_t = apool.tile([128, IC, cap], F16, tag="a")
                for ihalf in range(2):
                    wg_t = wpool.tile([128, HC, 512], F16, tag="w")
                    nc.sync.dma_start(
                        wg_t[:],
                        wg_d.ap()[e][:, :, ihalf * 512:(ihalf + 1) * 512]
                        .rearrange("c p i -> p c i"))
                    for it in range(4):
                        ig = ihalf * 4 + it
                        psg = pp.tile([128, cap], F32, name="ps", tag="ps")
                        for hc in range(HC):
                            nc.tensor.matmul(
                                psg[:],
                                wg_t[:, hc, it * 128:(it + 1) * 128],
                                xe_t[:, hc, :],
                                start=(hc == 0), stop=(hc == HC - 1),
                            )
                        nc.scalar.activation(sg_t[:, ig, :], psg[:], ACT.Silu)
                    wu_t = wpool.tile([128, HC, 512], F16, tag="w")
                    nc.sync.dma_start(
                        wu_t[:],
                        wu_d.ap()[e][:, :, ihalf * 512:(ihalf + 1) * 512]
                        .rearrange("c p i -> p c i"))
                    for it in range(4):
                        ig = ihalf * 4 + it
                        psu = pp.tile([128, cap], F32, name="ps", tag="ps")
                        for hc in range(HC):
                            nc.tensor.matmul(
                                psu[:],
                                wu_t[:, hc, it * 128:(it + 1) * 128],
                                xe_t[:, hc, :],
                                start=(hc == 0), stop=(hc == HC - 1),
                            )
                        nc.vector.tensor_mul(
                            a_t[:, ig, :], sg_t[:, ig, :], psu[:])
                yo_e = opool.tile([128, HC, cap], F16, tag="o")
                for hh in range(2):
                    wd_t = wpool.tile([128, IC, 1024], F16, tag="w")
                    nc.sync.dma_start(
                        wd_t[:],
                        wd_d.ap()[e][:, :, hh * 1024:(hh + 1) * 1024]
                        .rearrange("c p i -> p c i"))
                    for ht in range(IC):
                        psy = pp.tile([128, cap], F32, name="ps", tag="ps")
                        for ic in range(IC):
                            nc.tensor.matmul(
                                psy[:],
                                wd_t[:, ic, ht * 128:(ht + 1) * 128],
                                a_t[:, ic, :],
                                start=(ic == 0), stop=(ic == IC - 1),
                            )
                        nc.vector.tensor_copy(yo_e[:, hh * 8 + ht, :], psy[:])
                nc.sync.dma_start(yr_d[e].ap(), yo_e[:])

    nc.compile()
    return nc


def kernel(hidden_states, gate_w, e_bias, w_gate, w_up, w_down,
           ws_gate, ws_up, ws_down):
    global _LAST_RESULT
    _install_ntff_shim()
    from concourse.bass_utils import run_bass_kernel_spmd

    x = np.ascontiguousarray(np.asarray(hidden_states, dtype=np.float32))
    gate_w = np.asarray(gate_w, dtype=np.float32)
    e_bias = np.asarray(e_bias, dtype=np.float32)
    w_gate = np.ascontiguousarray(np.asarray(w_gate, dtype=np.float32))
    w_up = np.ascontiguousarray(np.asarray(w_up, dtype=np.float32))
    w_down = np.ascontiguousarray(np.asarray(w_down, dtype=np.float32))
    ws_gate = np.ascontiguousarray(np.asarray(ws_gate, dtype=np.float32))
    ws_up = np.ascontiguousarray(np.asarray(ws_up, dtype=np.float32))
    ws_down = np.ascontiguousarray(np.asarray(ws_down, dtype=np.float32))

    w_route, idx = _routing(x, gate_w, e_bias)

    # per-expert token lists + per-slot routing weights
    tok = [np.nonzero((idx == e).any(axis=1))[0] for e in range(E)]
    wt = []
    for e in range(E):
        k_of_t = (idx[tok[e]] == e).argmax(axis=1)
        wt.append(w_route[tok[e], k_of_t])
    counts = np.array([len(t) for t in tok])
    if counts.max() > CMAX:
        raise ValueError(f"expert load {counts.max()} exceeds capacity {CMAX}")

    # serpentine count-ranked assignment: slot s of core c gets expert
    # perm[c][s]; slot capacities are uniform across cores.
    order = np.argsort(-counts, kind="stable")
    perm = np.zeros((N_CORES, EPC), np.int64)
    for s in range(EPC):
        grp = order[s * N_CORES:(s + 1) * N_CORES]
        perm[:, s] = grp if s % 2 == 0 else grp[::-1]
    caps = tuple(
        int(max(4, ((counts[perm[:, s]].max() + 3) // 4) * 4))
        for s in range(EPC)
    )

    if caps not in _KERNEL_CACHE:
        _KERNEL_CACHE[caps] = _build_kernel(caps)
    nc = _KERNEL_CACHE[caps]

    x16 = x.astype(np.float16)
    xt_l = np.ascontiguousarray(x16.T).reshape(HC, 128, T)
    in_maps = []
    for c in range(N_CORES):
        es = perm[c]
        in_map = {"xt": xt_l}
        for s in range(EPC):
            e = es[s]
            buf = np.zeros((caps[s], H), np.float16)
            buf[: len(tok[e])] = x16[tok[e]]
            # [cap, H] -> [H, cap] -> [HC, 128, cap] -> partition-major
            in_map[f"xe{s}"] = np.ascontiguousarray(
                buf.T.reshape(HC, 128, caps[s]).transpose(1, 0, 2))
        in_map["wg"] = np.ascontiguousarray(
            w_gate[es].astype(np.float16)).reshape(EPC, HC, 128, I_DIM)
        in_map["wu"] = np.ascontiguousarray(
            w_up[es].astype(np.float16)).reshape(EPC, HC, 128, I_DIM)
        in_map["wd"] = np.ascontiguousarray(
            w_down[es].astype(np.float16)).reshape(EPC, IC, 128, H)
        in_map["wsg"] = np.ascontiguousarray(
            ws_gate[:, c * ISC:(c + 1) * ISC].astype(np.float16)).reshape(
                HC, 128, ISC)
        in_map["wsu"] = np.ascontiguousarray(
            ws_up[:, c * ISC:(c + 1) * ISC].astype(np.float16)).reshape(
                HC, 128, ISC)
        in_map["wsd"] = np.ascontiguousarray(
            ws_down[c * ISC:(c + 1) * ISC].astype(np.float16)).reshape(
                ISC // 128, 128, H)
        in_maps.append(in_map)

    try:
        res = run_bass_kernel_spmd(nc, in_maps,
                                   core_ids=list(range(N_CORES)))
    except Exception:
        res = run_bass_kernel_spmd(nc, in_maps,
                                   core_ids=list(range(N_CORES)))
    _LAST_RESULT = res

    y = np.zeros((H, T), np.float32)
    for c in range(N_CORES):
        y += res.results[c]["ys"].reshape(H, T).astype(np.float32)
    out = np.ascontiguousarray(y.T)
    for c in range(N_CORES):
        for s in range(EPC):
            e = perm[c][s]
            cnt = len(tok[e])
            if cnt == 0:
                continue
            yr = res.results[c][f"yr{s}"].astype(np.float32)
            O = yr.transpose(1, 0, 2).reshape(H, caps[s])[:, :cnt]
            out[tok[e]] += wt[e][:, None] * O.T
    return out


# revision 14
# speedup vs baseline: 1.0507x; 1.0507x over previous
"""ExaoneMoESparseMoEBlock Trainium2 kernel.

Strategy (expert-parallel over 8 NeuronCores):
  - Routing (gate matmul + biased grouped top-k) computed host-side in float64
    (selection margins >> fp32 noise, matches the fp32 jax reference).
  - Tokens are dispatched host-side. Experts are assigned to (core, slot) by
    token-count rank in a serpentine order so that every core's slot s holds a
    similarly-loaded expert: slot capacities are uniform across cores (SPMD)
    and per-core work is balanced.
  - Each core runs the SiLU-gated MLP for its 8 experts over their gathered
    tokens in a transposed layout ([feature, token]) — weights stationary,
    tokens moving — so no on-device transposes are needed.
  - Matmul operands are fp16 (1 cyc/row at any N, ~10-bit mantissa; weights
    have sigma=0.02 so fp16 quantization error ~5e-4 relative), accumulation
    in fp32 PSUM. fp16 halves the weight-streaming bytes, which is the
    bandwidth floor of this kernel. Matmuls run at exact per-slot token
    widths. Expert outputs are written back in fp16 and upcast on host.
  - Weights stream in half-expert tiles (16KB/partition) through an 8-deep
    pool so the DMA queue never stalls on a slot release.
  - The shared expert (IS=2048) is tensor-parallel sharded over the 8 cores
    (256 inter-dim slice each); each core emits a full [H, T] partial.
  - Host applies routing weights, scatter-adds expert outputs, and sums the
    shared partials.
"""

import sys
import types

import numpy as np

T, H, E, K_TOP = 1024, 2048, 64, 8
G, TG = 8, 4
I_DIM, IS_DIM = 1024, 2048
SCALE = 2.5
N_CORES = 8
EPC = E // N_CORES       # experts per core
ISC = IS_DIM // N_CORES  # shared-expert intermediate slice per core
HC = H // 128            # 16 h-chunks
IC = I_DIM // 128        # 8 i-chunks
CMAX = 512               # hard per-expert capacity limit (moving-dim max)

_LAST_RESULT = None      # BassKernelResults of the most recent run (for test.py)


def _install_ntff_shim():
    """Register the axon NTFF profile hook if the image's antenv lacks it.

    Lets BASS_TRACE=1 produce a perfetto trace + exec_time_ns. Harmless no-op
    when tracing is off or the axon .so is absent.
    """
    try:
        import antenv
        if "antenv.axon_hooks" in sys.modules:
            return
        mod = types.ModuleType("antenv.axon_hooks")
        mod._hook = None
        mod.set_axon_ntff_profile_hook = lambda h: setattr(mod, "_hook", h)
        mod.get_axon_ntff_profile_hook = lambda: mod._hook
        sys.modules["antenv.axon_hooks"] = mod
        antenv.axon_hooks = mod
        from trn_agent_boot.trn_boot import _ntff_profile_via_ctypes
        mod.set_axon_ntff_profile_hook(
            _ntff_profile_via_ctypes("/opt/axon/libaxon_pjrt.so")
        )
    except Exception:
        pass


def _routing(x, gate_w, e_bias):
    """float64 replica of the reference's sigmoid biased grouped top-k."""
    logits = x.astype(np.float64) @ gate_w.astype(np.float64)
    scores = 1.0 / (1.0 + np.exp(-logits))
    sb = scores + e_bias.astype(np.float64)[None, :]
    gsz = E // G
    gs = sb.reshape(T, G, gsz)
    top2 = np.sort(gs, axis=-1)[:, :, -2:].sum(-1)
    gidx = np.argsort(-top2, axis=-1, kind="stable")[:, :TG]
    gmask = np.zeros((T, G), bool)
    gmask[np.arange(T)[:, None], gidx] = True
    masked = np.where(np.repeat(gmask, gsz, axis=1), sb, -np.inf)
    idx = np.argsort(-masked, axis=-1, kind="stable")[:, :K_TOP]
    w = np.take_along_axis(scores, idx, axis=1).astype(np.float32)
    w = w / w.sum(-1, keepdims=True)
    return (w * np.float32(SCALE)).astype(np.float32), idx.astype(np.int64)


_KERNEL_CACHE = {}


def _build_kernel(caps):
    """Per-core SPMD Bass program. caps[s] = token columns of expert slot s."""
    from concourse import bacc
    import concourse.mybir as mybir
    import concourse.tile as tile

    F32 = mybir.dt.float32
    F16 = mybir.dt.float16
    ACT = mybir.ActivationFunctionType

    nc = bacc.Bacc("TRN2", target_bir_lowering=False, debug=False)

    slots = len(caps)
    xe_d = [nc.dram_tensor(f"xe{s}", [128, HC, caps[s]], F16,
                           kind="ExternalInput") for s in range(slots)]
    wg_d = nc.dram_tensor("wg", [slots, HC, 128, I_DIM], F16, kind="ExternalInput")
    wu_d = nc.dram_tensor("wu", [slots, HC, 128, I_DIM], F16, kind="ExternalInput")
    wd_d = nc.dram_tensor("wd", [slots, IC, 128, H], F16, kind="ExternalInput")
    xt_d = nc.dram_tensor("xt", [HC, 128, T], F16, kind="ExternalInput")
    wsg_d = nc.dram_tensor("wsg", [HC, 128, ISC], F16, kind="ExternalInput")
    wsu_d = nc.dram_tensor("wsu", [HC, 128, ISC], F16, kind="ExternalInput")
    wsd_d = nc.dram_tensor("wsd", [ISC // 128, 128, H], F16, kind="ExternalInput")
    yr_d = [nc.dram_tensor(f"yr{s}", [128, HC, caps[s]], F16,
                           kind="ExternalOutput") for s in range(slots)]
    ys_d = nc.dram_tensor("ys", [HC, 128, T], F16, kind="ExternalOutput")

    with tile.TileContext(nc) as tc:
        with (
            tc.tile_pool(name="wpool", bufs=8) as wpool,     # 16KB/part slots
            tc.tile_pool(name="xpool", bufs=2) as xpool,
            tc.tile_pool(name="sgpool", bufs=2) as sgpool,
            tc.tile_pool(name="apool", bufs=2) as apool,
            tc.tile_pool(name="opool", bufs=2) as opool,
            tc.tile_pool(name="xtpool", bufs=3) as xtpool,
            tc.tile_pool(name="pp", bufs=8, space="PSUM") as pp,
        ):
            # ------------- shared expert (TP slice of IS), single x pass -----
            wsg_t = wpool.tile([128, HC, ISC], F16, tag="w")
            nc.sync.dma_start(wsg_t[:], wsg_d.ap().rearrange("c p i -> p c i"))
            wsu_t = wpool.tile([128, HC, ISC], F16, tag="w")
            nc.sync.dma_start(wsu_t[:], wsu_d.ap().rearrange("c p i -> p c i"))
            psg_s = [pp.tile([128, 512], F32, name="ps", tag="ps")
                     for _ in range(4)]
            psu_s = [pp.tile([128, 512], F32, name="ps", tag="ps")
                     for _ in range(4)]
            for hc in range(HC):
                xt_t = xtpool.tile([128, T], F16)
                nc.sync.dma_start(xt_t[:], xt_d.ap()[hc])
                for it in range(2):
                    for nh in range(2):
                        nc.tensor.matmul(
                            psg_s[2 * it + nh][:],
                            wsg_t[:, hc, it * 128:(it + 1) * 128],
                            xt_t[:, nh * 512:(nh + 1) * 512],
                            start=(hc == 0), stop=(hc == HC - 1),
                        )
                        nc.tensor.matmul(
                            psu_s[2 * it + nh][:],
                            wsu_t[:, hc, it * 128:(it + 1) * 128],
                            xt_t[:, nh * 512:(nh + 1) * 512],
                            start=(hc == 0), stop=(hc == HC - 1),
                        )
            sg_s = sgpool.tile([128, 2, T], F32, tag="sg")
            sa_s = apool.tile([128, 2, T], F16, tag="a")
            for it in range(2):
                for nh in range(2):
                    sl = slice(nh * 512, (nh + 1) * 512)
                    nc.scalar.activation(
                        sg_s[:, it, sl], psg_s[2 * it + nh][:], ACT.Silu)
                    nc.vector.tensor_mul(
                        sa_s[:, it, sl], sg_s[:, it, sl], psu_s[2 * it + nh][:])

            wsd_t = wpool.tile([128, ISC // 128, H], F16, tag="w")
            nc.sync.dma_start(wsd_t[:], wsd_d.ap().rearrange("c p i -> p c i"))
            for htg in range(4):
                yo = opool.tile([128, 4, T], F16, tag="o")
                for hi in range(4):
                    ht = htg * 4 + hi
                    psy_s = [pp.tile([128, 512], F32, name="ps", tag="ps")
                             for _ in range(2)]
                    for ic in range(2):
                        for nh in range(2):
                            nc.tensor.matmul(
                                psy_s[nh][:],
                                wsd_t[:, ic, ht * 128:(ht + 1) * 128],
                                sa_s[:, ic, nh * 512:(nh + 1) * 512],
                                start=(ic == 0), stop=(ic == 1),
                            )
                    nc.vector.tensor_copy(yo[:, hi, 0:512], psy_s[0][:])
                    nc.vector.tensor_copy(yo[:, hi, 512:1024], psy_s[1][:])
                nc.sync.dma_start(
                    ys_d.ap().rearrange("c p t -> p c t")[:, htg * 4:(htg + 1) * 4, :],
                    yo[:])

            # ------------- routed expert shards -------------
            for e in range(slots):
                cap = caps[e]
                xe_t = xpool.tile([128, HC, cap], F16, tag="xe")
                nc.sync.dma_start(xe_t[:], xe_d[e].ap())
                sg_t = sgpool.tile([128, IC, cap], F32, tag="sg")
                a_t = apool.tile([128, IC, cap], F16, tag="a")
                for ihalf in range(2):
                    wg_t = wpool.tile([128, HC, 512], F16, tag="w")
                    nc.sync.dma_start(
                        wg_t[:],
                        wg_d.ap()[e][:, :, ihalf * 512:(ihalf + 1) * 512]
                        .rearrange("c p i -> p c i"))
                    for it in range(4):
                        ig = ihalf * 4 + it
                        psg = pp.tile([128, cap], F32, name="ps", tag="ps")
                        for hc in range(HC):
                            nc.tensor.matmul(
                                psg[:],
                                wg_t[:, hc, it * 128:(it + 1) * 128],
                                xe_t[:, hc, :],
                                start=(hc == 0), stop=(hc == HC - 1),
                            )
                        nc.scalar.activation(sg_t[:, ig, :], psg[:], ACT.Silu)
                    wu_t = wpool.tile([128, HC, 512], F16, tag="w")
                    nc.sync.dma_start(
                        wu_t[:],
                        wu_d.ap()[e][:, :, ihalf * 512:(ihalf + 1) * 512]
                        .rearrange("c p i -> p c i"))
                    for it in range(4):
                        ig = ihalf * 4 + it
                        psu = pp.tile([128, cap], F32, name="ps", tag="ps")
                        for hc in range(HC):
                            nc.tensor.matmul(
                                psu[:],
                                wu_t[:, hc, it * 128:(it + 1) * 128],
                                xe_t[:, hc, :],
                                start=(hc == 0), stop=(hc == HC - 1),
                            )
                        nc.vector.tensor_mul(
                            a_t[:, ig, :], sg_t[:, ig, :], psu[:])
                yo_e = opool.tile([128, HC, cap], F16, tag="o")
                for hh in range(2):
                    wd_t = wpool.tile([128, IC, 1024], F16, tag="w")
                    nc.sync.dma_start(
                        wd_t[:],
                        wd_d.ap()[e][:, :, hh * 1024:(hh + 1) * 1024]
                        .rearrange("c p i -> p c i"))
                    for ht in range(IC):
                        psy = pp.tile([128, cap], F32, name="ps", tag="ps")
                        for ic in range(IC):
                            nc.tensor.matmul(
                                psy[:],
                                wd_t[:, ic, ht * 128:(ht + 1) * 128],
                                a_t[:, ic, :],
                                start=(ic == 0), stop=(ic == IC - 1),
                            )
                        nc.vector.tensor_copy(yo_e[:, hh * 8 + ht, :], psy[:])
                nc.sync.dma_start(yr_d[e].ap(), yo_e[:])

    nc.compile()
    return nc


def kernel(hidden_states, gate_w, e_bias, w_gate, w_up, w_down,
           ws_gate, ws_up, ws_down):
    global _LAST_RESULT
    _install_ntff_shim()
    from concourse.bass_utils import run_bass_kernel_spmd

    x = np.ascontiguousarray(np.asarray(hidden_states, dtype=np.float32))
    gate_w = np.asarray(gate_w, dtype=np.float32)
    e_bias = np.asarray(e_bias, dtype=np.float32)
    w_gate = np.ascontiguousarray(np.asarray(w_gate, dtype=np.float32))
    w_up = np.ascontiguousarray(np.asarray(w_up, dtype=np.float32))
    w_down = np.ascontiguousarray(np.asarray(w_down, dtype=np.float32))
    ws_gate = np.ascontiguousarray(np.asarray(ws_gate, dtype=np.float32))
    ws_up = np.ascontiguousarray(np.asarray(ws_up, dtype=np.float32))
    ws_down = np.ascontiguousarray(np.asarray(ws_down, dtype=np.float32))

    w_route, idx = _routing(x, gate_w, e_bias)

    # per-expert token lists + per-token routing weights; experts with more
    # than CMAX tokens are split into multiple shards, empty experts dropped
    shards = []  # (expert_id, token_ids, weights)
    for e in range(E):
        te = np.nonzero((idx == e).any(axis=1))[0]
        if len(te) == 0:
            continue
        k_of_t = (idx[te] == e).argmax(axis=1)
        we = w_route[te, k_of_t]
        for s0 in range(0, len(te), CMAX):
            shards.append((e, te[s0:s0 + CMAX], we[s0:s0 + CMAX]))
    while len(shards) % N_CORES != 0:
        shards.append((0, np.zeros(0, np.int64), np.zeros(0, np.float32)))
    n_slots = len(shards) // N_CORES

    # serpentine count-ranked assignment: slot s of core c gets shard
    # perm[c][s]; slot capacities are uniform across cores.
    scounts = np.array([len(s[1]) for s in shards])
    order = np.argsort(-scounts, kind="stable")
    perm = np.zeros((N_CORES, n_slots), np.int64)
    for s in range(n_slots):
        grp = order[s * N_CORES:(s + 1) * N_CORES]
        perm[:, s] = grp if s % 2 == 0 else grp[::-1]
    caps = tuple(
        int(max(4, ((scounts[perm[:, s]].max() + 3) // 4) * 4))
        for s in range(n_slots)
    )

    if caps not in _KERNEL_CACHE:
        _KERNEL_CACHE[caps] = _build_kernel(caps)
    nc = _KERNEL_CACHE[caps]

    x16 = x.astype(np.float16)
    xt_l = np.ascontiguousarray(x16.T).reshape(HC, 128, T)
    in_maps = []
    for c in range(N_CORES):
        sh = [shards[j] for j in perm[c]]
        es = np.array([s[0] for s in sh])
        in_map = {"xt": xt_l}
        for s in range(n_slots):
            te = sh[s][1]
            buf = np.zeros((caps[s], H), np.float16)
            buf[: len(te)] = x16[te]
            # [cap, H] -> [H, cap] -> [HC, 128, cap] -> partition-major
            in_map[f"xe{s}"] = np.ascontiguousarray(
                buf.T.reshape(HC, 128, caps[s]).transpose(1, 0, 2))
        in_map["wg"] = np.ascontiguousarray(
            w_gate[es].astype(np.float16)).reshape(n_slots, HC, 128, I_DIM)
        in_map["wu"] = np.ascontiguousarray(
            w_up[es].astype(np.float16)).reshape(n_slots, HC, 128, I_DIM)
        in_map["wd"] = np.ascontiguousarray(
            w_down[es].astype(np.float16)).reshape(n_slots, IC, 128, H)
        in_map["wsg"] = np.ascontiguousarray(
            ws_gate[:, c * ISC:(c + 1) * ISC].astype(np.float16)).reshape(
                HC, 128, ISC)
        in_map["wsu"] = np.ascontiguousarray(
            ws_up[:, c * ISC:(c + 1) * ISC].astype(np.float16)).reshape(
                HC, 128, ISC)
        in_map["wsd"] = np.ascontiguousarray(
            ws_down[c * ISC:(c + 1) * ISC].astype(np.float16)).reshape(
                ISC // 128, 128, H)
        in_maps.append(in_map)

    try:
        res = run_bass_kernel_spmd(nc, in_maps,
                                   core_ids=list(range(N_CORES)))
    except Exception:
        res = run_bass_kernel_spmd(nc, in_maps,
                                   core_ids=list(range(N_CORES)))
    _LAST_RESULT = res

    y = np.zeros((H, T), np.float32)
    for c in range(N_CORES):
        y += res.results[c]["ys"].reshape(H, T).astype(np.float32)
    out = np.ascontiguousarray(y.T)
    for c in range(N_CORES):
        for s in range(n_slots):
            _, te, we = shards[perm[c][s]]
            cnt = len(te)
            if cnt == 0:
                continue
            yr = res.results[c][f"yr{s}"].astype(np.float32)
            O = yr.transpose(1, 0, 2).reshape(H, caps[s])[:, :cnt]
            out[te] += we[:, None] * O.T
    return out


# revision 15
# speedup vs baseline: 1.0793x; 1.0273x over previous
"""ExaoneMoESparseMoEBlock Trainium2 kernel.

Strategy (expert-parallel over 8 NeuronCores):
  - Routing (gate matmul + biased grouped top-k) computed host-side in float64
    (selection margins >> fp32 noise, matches the fp32 jax reference).
  - Tokens are dispatched host-side. Experts are assigned to (core, slot) by
    token-count rank in a serpentine order so that every core's slot s holds a
    similarly-loaded expert: slot capacities are uniform across cores (SPMD)
    and per-core work is balanced.
  - Each core runs the SiLU-gated MLP for its 8 experts over their gathered
    tokens in a transposed layout ([feature, token]) — weights stationary,
    tokens moving — so no on-device transposes are needed.
  - Matmul operands are fp16 (1 cyc/row at any N, ~10-bit mantissa; weights
    have sigma=0.02 so fp16 quantization error ~5e-4 relative), accumulation
    in fp32 PSUM. fp16 halves the weight-streaming bytes, which is the
    bandwidth floor of this kernel. Matmuls run at exact per-slot token
    widths. Expert outputs are written back in fp16 and upcast on host.
  - Weights stream in half-expert tiles (16KB/partition) through an 8-deep
    pool so the DMA queue never stalls on a slot release.
  - The shared expert (IS=2048) is tensor-parallel sharded over the 8 cores
    (256 inter-dim slice each); each core emits a full [H, T] partial.
  - Host applies routing weights, scatter-adds expert outputs, and sums the
    shared partials.
"""

import sys
import types

import numpy as np

T, H, E, K_TOP = 1024, 2048, 64, 8
G, TG = 8, 4
I_DIM, IS_DIM = 1024, 2048
SCALE = 2.5
N_CORES = 8
EPC = E // N_CORES       # experts per core
ISC = IS_DIM // N_CORES  # shared-expert intermediate slice per core
HC = H // 128            # 16 h-chunks
IC = I_DIM // 128        # 8 i-chunks
CMAX = 512               # hard per-expert capacity limit (moving-dim max)

_LAST_RESULT = None      # BassKernelResults of the most recent run (for test.py)


def _install_ntff_shim():
    """Register the axon NTFF profile hook if the image's antenv lacks it.

    Lets BASS_TRACE=1 produce a perfetto trace + exec_time_ns. Harmless no-op
    when tracing is off or the axon .so is absent.
    """
    try:
        import antenv
        if "antenv.axon_hooks" in sys.modules:
            return
        mod = types.ModuleType("antenv.axon_hooks")
        mod._hook = None
        mod.set_axon_ntff_profile_hook = lambda h: setattr(mod, "_hook", h)
        mod.get_axon_ntff_profile_hook = lambda: mod._hook
        sys.modules["antenv.axon_hooks"] = mod
        antenv.axon_hooks = mod
        from trn_agent_boot.trn_boot import _ntff_profile_via_ctypes
        mod.set_axon_ntff_profile_hook(
            _ntff_profile_via_ctypes("/opt/axon/libaxon_pjrt.so")
        )
    except Exception:
        pass


def _routing(x, gate_w, e_bias):
    """float64 replica of the reference's sigmoid biased grouped top-k."""
    logits = x.astype(np.float64) @ gate_w.astype(np.float64)
    scores = 1.0 / (1.0 + np.exp(-logits))
    sb = scores + e_bias.astype(np.float64)[None, :]
    gsz = E // G
    gs = sb.reshape(T, G, gsz)
    top2 = np.sort(gs, axis=-1)[:, :, -2:].sum(-1)
    gidx = np.argsort(-top2, axis=-1, kind="stable")[:, :TG]
    gmask = np.zeros((T, G), bool)
    gmask[np.arange(T)[:, None], gidx] = True
    masked = np.where(np.repeat(gmask, gsz, axis=1), sb, -np.inf)
    idx = np.argsort(-masked, axis=-1, kind="stable")[:, :K_TOP]
    w = np.take_along_axis(scores, idx, axis=1).astype(np.float32)
    w = w / w.sum(-1, keepdims=True)
    return (w * np.float32(SCALE)).astype(np.float32), idx.astype(np.int64)


_KERNEL_CACHE = {}


def _build_kernel(caps):
    """Per-core SPMD Bass program. caps[s] = token columns of expert slot s."""
    from concourse import bacc
    import concourse.mybir as mybir
    import concourse.tile as tile

    F32 = mybir.dt.float32
    F16 = mybir.dt.float16
    ACT = mybir.ActivationFunctionType

    nc = bacc.Bacc("TRN2", target_bir_lowering=False, debug=False)

    slots = len(caps)
    xe_d = [nc.dram_tensor(f"xe{s}", [128, HC, caps[s]], F16,
                           kind="ExternalInput") for s in range(slots)]
    wg_d = nc.dram_tensor("wg", [slots, HC, 128, I_DIM], F16, kind="ExternalInput")
    wu_d = nc.dram_tensor("wu", [slots, HC, 128, I_DIM], F16, kind="ExternalInput")
    wd_d = nc.dram_tensor("wd", [slots, IC, 128, H], F16, kind="ExternalInput")
    xt_d = nc.dram_tensor("xt", [HC, 128, T], F16, kind="ExternalInput")
    wsg_d = nc.dram_tensor("wsg", [HC, 128, ISC], F16, kind="ExternalInput")
    wsu_d = nc.dram_tensor("wsu", [HC, 128, ISC], F16, kind="ExternalInput")
    wsd_d = nc.dram_tensor("wsd", [ISC // 128, 128, H], F16, kind="ExternalInput")
    yr_d = [nc.dram_tensor(f"yr{s}", [128, HC, caps[s]], F16,
                           kind="ExternalOutput") for s in range(slots)]
    ys_d = nc.dram_tensor("ys", [HC, 128, T], F16, kind="ExternalOutput")

    # adaptive weight-pool depth: large token capacities grow the xe/sg/a/o
    # slots, so shrink the 16KB-slot weight pipeline to fit 192KB/partition
    capmax = max(caps)
    other_kb = (2 * (HC * capmax * 2) + 2 * max(IC * capmax * 4, 8192)
                + 2 * max(IC * capmax * 2, 4096)
                + 2 * max(HC * capmax * 2, 8192) + 3 * 2048) / 1024.0
    wbufs = int(max(2, min(8, (192 - other_kb - 6) // 16)))

    with tile.TileContext(nc) as tc:
        with (
            tc.tile_pool(name="wpool", bufs=wbufs) as wpool,  # 16KB/part slots
            tc.tile_pool(name="xpool", bufs=2) as xpool,
            tc.tile_pool(name="sgpool", bufs=2) as sgpool,
            tc.tile_pool(name="apool", bufs=2) as apool,
            tc.tile_pool(name="opool", bufs=2) as opool,
            tc.tile_pool(name="xtpool", bufs=3) as xtpool,
            tc.tile_pool(name="pp", bufs=8, space="PSUM") as pp,
        ):
            # ------------- shared expert (TP slice of IS), single x pass -----
            wsg_t = wpool.tile([128, HC, ISC], F16, tag="w")
            nc.sync.dma_start(wsg_t[:], wsg_d.ap().rearrange("c p i -> p c i"))
            wsu_t = wpool.tile([128, HC, ISC], F16, tag="w")
            nc.sync.dma_start(wsu_t[:], wsu_d.ap().rearrange("c p i -> p c i"))
            psg_s = [pp.tile([128, 512], F32, name="ps", tag="ps")
                     for _ in range(4)]
            psu_s = [pp.tile([128, 512], F32, name="ps", tag="ps")
                     for _ in range(4)]
            for hc in range(HC):
                xt_t = xtpool.tile([128, T], F16)
                nc.sync.dma_start(xt_t[:], xt_d.ap()[hc])
                for it in range(2):
                    for nh in range(2):
                        nc.tensor.matmul(
                            psg_s[2 * it + nh][:],
                            wsg_t[:, hc, it * 128:(it + 1) * 128],
                            xt_t[:, nh * 512:(nh + 1) * 512],
                            start=(hc == 0), stop=(hc == HC - 1),
                        )
                        nc.tensor.matmul(
                            psu_s[2 * it + nh][:],
                            wsu_t[:, hc, it * 128:(it + 1) * 128],
                            xt_t[:, nh * 512:(nh + 1) * 512],
                            start=(hc == 0), stop=(hc == HC - 1),
                        )
            sg_s = sgpool.tile([128, 2, T], F32, tag="sg")
            sa_s = apool.tile([128, 2, T], F16, tag="a")
            for it in range(2):
                for nh in range(2):
                    sl = slice(nh * 512, (nh + 1) * 512)
                    nc.scalar.activation(
                        sg_s[:, it, sl], psg_s[2 * it + nh][:], ACT.Silu)
                    nc.vector.tensor_mul(
                        sa_s[:, it, sl], sg_s[:, it, sl], psu_s[2 * it + nh][:])

            wsd_t = wpool.tile([128, ISC // 128, H], F16, tag="w")
            nc.sync.dma_start(wsd_t[:], wsd_d.ap().rearrange("c p i -> p c i"))
            for htg in range(4):
                yo = opool.tile([128, 4, T], F16, tag="o")
                for hi in range(4):
                    ht = htg * 4 + hi
                    psy_s = [pp.tile([128, 512], F32, name="ps", tag="ps")
                             for _ in range(2)]
                    for ic in range(2):
                        for nh in range(2):
                            nc.tensor.matmul(
                                psy_s[nh][:],
                                wsd_t[:, ic, ht * 128:(ht + 1) * 128],
                                sa_s[:, ic, nh * 512:(nh + 1) * 512],
                                start=(ic == 0), stop=(ic == 1),
                            )
                    nc.vector.tensor_copy(yo[:, hi, 0:512], psy_s[0][:])
                    nc.vector.tensor_copy(yo[:, hi, 512:1024], psy_s[1][:])
                nc.sync.dma_start(
                    ys_d.ap().rearrange("c p t -> p c t")[:, htg * 4:(htg + 1) * 4, :],
                    yo[:])

            # ------------- routed expert shards -------------
            for e in range(slots):
                cap = caps[e]
                xe_t = xpool.tile([128, HC, cap], F16, tag="xe")
                nc.sync.dma_start(xe_t[:], xe_d[e].ap())
                sg_t = sgpool.tile([128, IC, cap], F32, tag="sg")
                a_t = apool.tile([128, IC, cap], F16, tag="a")
                for ihalf in range(2):
                    wg_t = wpool.tile([128, HC, 512], F16, tag="w")
                    nc.sync.dma_start(
                        wg_t[:],
                        wg_d.ap()[e][:, :, ihalf * 512:(ihalf + 1) * 512]
                        .rearrange("c p i -> p c i"))
                    for it in range(4):
                        ig = ihalf * 4 + it
                        psg = pp.tile([128, cap], F32, name="ps", tag="ps")
                        for hc in range(HC):
                            nc.tensor.matmul(
                                psg[:],
                                wg_t[:, hc, it * 128:(it + 1) * 128],
                                xe_t[:, hc, :],
                                start=(hc == 0), stop=(hc == HC - 1),
                            )
                        nc.scalar.activation(sg_t[:, ig, :], psg[:], ACT.Silu)
                    wu_t = wpool.tile([128, HC, 512], F16, tag="w")
                    nc.sync.dma_start(
                        wu_t[:],
                        wu_d.ap()[e][:, :, ihalf * 512:(ihalf + 1) * 512]
                        .rearrange("c p i -> p c i"))
                    for it in range(4):
                        ig = ihalf * 4 + it
                        psu = pp.tile([128, cap], F32, name="ps", tag="ps")
                        for hc in range(HC):
                            nc.tensor.matmul(
                                psu[:],
                                wu_t[:, hc, it * 128:(it + 1) * 128],
                                xe_t[:, hc, :],
                                start=(hc == 0), stop=(hc == HC - 1),
                            )
                        nc.vector.tensor_mul(
                            a_t[:, ig, :], sg_t[:, ig, :], psu[:])
                yo_e = opool.tile([128, HC, cap], F16, tag="o")
                for hh in range(2):
                    wd_t = wpool.tile([128, IC, 1024], F16, tag="w")
                    nc.sync.dma_start(
                        wd_t[:],
                        wd_d.ap()[e][:, :, hh * 1024:(hh + 1) * 1024]
                        .rearrange("c p i -> p c i"))
                    for ht in range(IC):
                        psy = pp.tile([128, cap], F32, name="ps", tag="ps")
                        for ic in range(IC):
                            nc.tensor.matmul(
                                psy[:],
                                wd_t[:, ic, ht * 128:(ht + 1) * 128],
                                a_t[:, ic, :],
                                start=(ic == 0), stop=(ic == IC - 1),
                            )
                        nc.vector.tensor_copy(yo_e[:, hh * 8 + ht, :], psy[:])
                nc.sync.dma_start(yr_d[e].ap(), yo_e[:])

    nc.compile()
    return nc


def kernel(hidden_states, gate_w, e_bias, w_gate, w_up, w_down,
           ws_gate, ws_up, ws_down):
    global _LAST_RESULT
    _install_ntff_shim()
    from concourse.bass_utils import run_bass_kernel_spmd

    x = np.ascontiguousarray(np.asarray(hidden_states, dtype=np.float32))
    gate_w = np.asarray(gate_w, dtype=np.float32)
    e_bias = np.asarray(e_bias, dtype=np.float32)
    w_gate = np.ascontiguousarray(np.asarray(w_gate, dtype=np.float32))
    w_up = np.ascontiguousarray(np.asarray(w_up, dtype=np.float32))
    w_down = np.ascontiguousarray(np.asarray(w_down, dtype=np.float32))
    ws_gate = np.ascontiguousarray(np.asarray(ws_gate, dtype=np.float32))
    ws_up = np.ascontiguousarray(np.asarray(ws_up, dtype=np.float32))
    ws_down = np.ascontiguousarray(np.asarray(ws_down, dtype=np.float32))

    w_route, idx = _routing(x, gate_w, e_bias)

    # per-expert token lists + per-token routing weights; experts with more
    # than CMAX tokens are split into multiple shards, empty experts dropped
    shards = []  # (expert_id, token_ids, weights)
    for e in range(E):
        te = np.nonzero((idx == e).any(axis=1))[0]
        if len(te) == 0:
            continue
        k_of_t = (idx[te] == e).argmax(axis=1)
        we = w_route[te, k_of_t]
        for s0 in range(0, len(te), CMAX):
            shards.append((e, te[s0:s0 + CMAX], we[s0:s0 + CMAX]))
    while len(shards) % N_CORES != 0:
        shards.append((0, np.zeros(0, np.int64), np.zeros(0, np.float32)))
    n_slots = len(shards) // N_CORES

    # serpentine count-ranked assignment: slot s of core c gets shard
    # perm[c][s]; slot capacities are uniform across cores.
    scounts = np.array([len(s[1]) for s in shards])
    order = np.argsort(-scounts, kind="stable")
    perm = np.zeros((N_CORES, n_slots), np.int64)
    for s in range(n_slots):
        grp = order[s * N_CORES:(s + 1) * N_CORES]
        perm[:, s] = grp if s % 2 == 0 else grp[::-1]
    caps = tuple(
        int(max(4, ((scounts[perm[:, s]].max() + 3) // 4) * 4))
        for s in range(n_slots)
    )

    if caps not in _KERNEL_CACHE:
        _KERNEL_CACHE[caps] = _build_kernel(caps)
    nc = _KERNEL_CACHE[caps]

    x16 = x.astype(np.float16)
    xt_l = np.ascontiguousarray(x16.T).reshape(HC, 128, T)
    in_maps = []
    for c in range(N_CORES):
        sh = [shards[j] for j in perm[c]]
        es = np.array([s[0] for s in sh])
        in_map = {"xt": xt_l}
        for s in range(n_slots):
            te = sh[s][1]
            buf = np.zeros((caps[s], H), np.float16)
            buf[: len(te)] = x16[te]
            # [cap, H] -> [H, cap] -> [HC, 128, cap] -> partition-major
            in_map[f"xe{s}"] = np.ascontiguousarray(
                buf.T.reshape(HC, 128, caps[s]).transpose(1, 0, 2))
        in_map["wg"] = np.ascontiguousarray(
            w_gate[es].astype(np.float16)).reshape(n_slots, HC, 128, I_DIM)
        in_map["wu"] = np.ascontiguousarray(
            w_up[es].astype(np.float16)).reshape(n_slots, HC, 128, I_DIM)
        in_map["wd"] = np.ascontiguousarray(
            w_down[es].astype(np.float16)).reshape(n_slots, IC, 128, H)
        in_map["wsg"] = np.ascontiguousarray(
            ws_gate[:, c * ISC:(c + 1) * ISC].astype(np.float16)).reshape(
                HC, 128, ISC)
        in_map["wsu"] = np.ascontiguousarray(
            ws_up[:, c * ISC:(c + 1) * ISC].astype(np.float16)).reshape(
                HC, 128, ISC)
        in_map["wsd"] = np.ascontiguousarray(
            ws_down[c * ISC:(c + 1) * ISC].astype(np.float16)).reshape(
                ISC // 128, 128, H)
        in_maps.append(in_map)

    try:
        res = run_bass_kernel_spmd(nc, in_maps,
                                   core_ids=list(range(N_CORES)))
    except Exception:
        res = run_bass_kernel_spmd(nc, in_maps,
                                   core_ids=list(range(N_CORES)))
    _LAST_RESULT = res

    y = np.zeros((H, T), np.float32)
    for c in range(N_CORES):
        y += res.results[c]["ys"].reshape(H, T).astype(np.float32)
    out = np.ascontiguousarray(y.T)
    for c in range(N_CORES):
        for s in range(n_slots):
            _, te, we = shards[perm[c][s]]
            cnt = len(te)
            if cnt == 0:
                continue
            yr = res.results[c][f"yr{s}"].astype(np.float32)
            O = yr.transpose(1, 0, 2).reshape(H, caps[s])[:, :cnt]
            out[te] += we[:, None] * O.T
    return out
